# revision 4
# baseline (speedup 1.0000x reference)
"""Block-diagonal GRU cell on 8 TRN2 NeuronCores — one block per core.

Math per block n (torch GRUCell):
  gi = x_n @ W_ih[n].T + b_ih[n]        (B, 3*BS)
  gh = h_n @ W_hh[n].T + b_hh[n]
  r = sigmoid(gi_r + gh_r); z = sigmoid(gi_z + gh_z)
  ng = tanh(gi_n + r * gh_n)
  h' = ng + z * (h_n - ng)

On-chip layout (per core): everything transposed on host so the
contraction (feature) dim is the SBUF partition dim and gates land on
PSUM partitions — biases then apply as per-partition ACT/DVE operands.
  A  = [W_ih[n].T ; W_hh[n].T]  -> (1024 feat, 1536 gates) in bf16,
       blocked per 128-gate column group, dram laid out partition-major
       so every slot-range load is one big contiguous-per-partition DMA.
  U  = [x_n.T ; h_n.T]          -> (1024 feat, 1024 batch) bf16
  out = h'.T                    -> (512, 1024) bf16, un-transposed and
       upcast on host.
All matmuls run in bf16 (full-rate PE, cheap LDWEIGHTS, half the HBM
traffic of fp32r). r/z gates accumulate x- and h-matmuls into one PSUM
bank (8 k-steps); the n gate keeps i_n / h_n in separate banks.
Combine avoids 1-z entirely: h' = nt - z*nt + z*h, with bf16
SBUF-resident DVE ops (2x dve mode) for everything not reading PSUM.
Loads are spread over three DGE queues (A on Sync, U on GpSimd SWDGE,
biases on Scalar) so trigger issue overhead doesn't serialize the fill;
short 128-wide PE warm-up matmuls bridge the fill for the p-state ramp.
"""

import os
import sys

import numpy as np

try:
    import concourse.bass as bass
except ImportError:  # fresh grading dir: fall back to the repo checkout
    sys.path.insert(0, "/opt/trn_rl_repo")
    import concourse.bass as bass

import concourse.mybir as mybir
import concourse.tile as tile
from concourse import bacc
from concourse.bass import ts
from concourse.bass_utils import run_bass_kernel_spmd

B = 1024            # batch
NB = 8              # blocks == cores
BS = 512            # hidden block size
G3 = 3 * BS         # gates per block (r, z, n)
KF = 1024           # contraction feats per core: 512 input + 512 hidden
P = 128
KT = KF // P        # 8 k-tiles
GT = G3 // P        # 12 gate column groups: 0-3 r, 4-7 z, 8-11 n
NBC = 2             # batch chunks
BC = B // NBC       # 512 (one PSUM bank of fp32)

F32 = mybir.dt.float32
BF16 = mybir.dt.bfloat16
AFT = mybir.ActivationFunctionType
ALU = mybir.AluOpType

_cache: dict = {}
LAST_RESULTS = None  # BassKernelResults of the most recent run (for test.py)


def _build_nc():
    nc = bacc.Bacc("TRN2", target_bir_lowering=False, debug=False, num_devices=NB)
    a_d = nc.dram_tensor("a", [P, GT * KT, P], BF16, kind="ExternalInput").ap()
    u_d = nc.dram_tensor("u", [P, KT, B], BF16, kind="ExternalInput").ap()
    brz_d = nc.dram_tensor("brz", [P, 12], F32, kind="ExternalInput").ap()
    bn_d = nc.dram_tensor("bn", [P, 8], F32, kind="ExternalInput").ap()
    o_d = nc.dram_tensor("o", [BS, B], BF16, kind="ExternalOutput").ap()

    with tile.TileContext(nc) as tc:
        with (
            tc.tile_pool(name="persist", bufs=1) as persist,
            tc.tile_pool(name="tmp", bufs=3) as tmp,
            tc.tile_pool(name="outp", bufs=4) as outp,
            tc.tile_pool(name="psum", bufs=8, space="PSUM") as psum,
        ):
            # tiny bias loads on the Scalar engine's queue, off the bulk paths
            brz_sb = persist.tile([P, 12], F32, name="brz_sb")
            nc.scalar.dma_start(brz_sb[:], brz_d[:])
            bn_sb = persist.tile([P, 8], F32, name="bn_sb")
            nc.scalar.dma_start(bn_sb[:], bn_d[:])

            U = persist.tile([P, KT, B], BF16, name="U")
            A = persist.tile([P, GT * KT, P], BF16, name="A")

            # PE warm-up scratch first on gpsimd so it's ready immediately
            wsb = persist.tile([P, P], BF16, name="wsb")
            nc.gpsimd.memset(wsb[:], 0.0)

            # Bulk loads, in consumption order. The Sync HWDGE queue starts
            # ~3us earlier than the GpSimd SWDGE queue, so everything the
            # first row-block group consumes rides Sync in exact order; the
            # late-needed bulk (U bc1, A slots for j>=1) rides GpSimd.
            def load_a(eng, s0, s1):
                eng.dma_start(
                    A[:, s0 * KT : s1 * KT, :], a_d[:, s0 * KT : s1 * KT, :]
                )

            def load_u(eng, k0, k1, bc):
                eng.dma_start(
                    U[:, k0:k1, ts(bc, BC)], u_d[:, k0:k1, ts(bc, BC)]
                )

            load_a(nc.sync, 0, 1)       # r0
            load_u(nc.sync, 0, 2, 0)
            load_u(nc.sync, 2, 4, 0)
            load_a(nc.sync, 1, 2)       # z0
            load_u(nc.sync, 4, 6, 0)
            load_u(nc.sync, 6, 8, 0)
            load_a(nc.sync, 2, 3)       # n0
            load_u(nc.gpsimd, 0, 8, 1)  # U bc1
            load_a(nc.gpsimd, 3, 6)     # j=1
            load_a(nc.gpsimd, 6, 9)     # j=2
            load_a(nc.gpsimd, 9, 12)    # j=3

            # PE warm-up: short 128-wide matmuls bridge the DMA fill so the
            # HAM activity window is hot when real matmuls start
            wps = psum.tile([P, BC], F32, name="wps", tag="ps")
            for _ in range(16):
                nc.tensor.matmul(wps[:, :P], wsb[:], wsb[:], start=True, stop=True)

            # logical gate group -> A slot: slot 3j=r_j (g=j), 3j+1=z_j
            # (g=4+j), 3j+2=n_j (g=8+j)
            def slot_of(g):
                j, kind = g % 4, g // 4
                return 3 * j + kind

            def lhsT(g, k):
                return A[:, slot_of(g) * KT + k, :]

            # persistent per row-block j: r gate, z gate, zh = z*h (bf16)
            r_t = [persist.tile([P, B], BF16, name=f"r{j}") for j in range(4)]
            z_t = [persist.tile([P, B], BF16, name=f"z{j}") for j in range(4)]
            zh = [persist.tile([P, B], BF16, name=f"zh{j}") for j in range(4)]

            def mm_group(g, c0, w, k0, k1):
                ps = psum.tile([P, w], F32, name="ps", tag="ps")
                for k in range(k0, k1):
                    nc.tensor.matmul(
                        ps[:],
                        lhsT(g, k),
                        U[:, k, c0 : c0 + w],
                        start=(k == k0),
                        stop=(k == k1 - 1),
                    )
                return ps

            def make_nt(j, c0, w, ps_i, ps_h, sl):
                # ng = tanh(i_n + b_in + r*(h_n + b_hn))
                t = tmp.tile([P, w], F32, name="t", tag="t")
                nc.vector.scalar_tensor_tensor(
                    t[:], ps_h[:, sl], bn_sb[:, 4 + j : 5 + j],
                    r_t[j][:, c0 : c0 + w], ALU.add, ALU.mult,
                )
                t2 = tmp.tile([P, w], BF16, name="t2", tag="t2")
                nc.vector.tensor_add(t2[:], t[:], ps_i[:, sl])
                nt = tmp.tile([P, w], BF16, name="nt", tag="nt")
                nc.scalar.activation(nt[:], t2[:], AFT.Tanh, bias=bn_sb[:, j : j + 1])
                return nt

            def combine(j, c0, w, nt, ntsl):
                # h' = ng - z*ng + z*h
                zn = tmp.tile([P, w], BF16, name="zn", tag="zn")
                nc.vector.tensor_mul(zn[:], z_t[j][:, c0 : c0 + w], nt[:, ntsl])
                m = tmp.tile([P, w], BF16, name="m", tag="m")
                nc.vector.tensor_sub(m[:], nt[:, ntsl], zn[:])
                o_t = outp.tile([P, w], BF16, name="o_t", tag="o_t")
                nc.vector.tensor_add(o_t[:], m[:], zh[j][:, c0 : c0 + w])
                nc.sync.dma_start(o_d[ts(j, P), c0 : c0 + w], o_t[:])

            for bc in range(NBC):
                for j in range(4):
                    last = bc == NBC - 1 and j == 3
                    ps_r = mm_group(j, bc * BC, BC, 0, KT)
                    nc.scalar.activation(
                        r_t[j][:, ts(bc, BC)], ps_r[:], AFT.Sigmoid,
                        bias=brz_sb[:, j : j + 1],
                    )
                    if not last:
                        ps_z = mm_group(4 + j, bc * BC, BC, 0, KT)
                        nc.scalar.activation(
                            z_t[j][:, ts(bc, BC)], ps_z[:], AFT.Sigmoid,
                            bias=brz_sb[:, 4 + j : 5 + j],
                        )
                        nc.vector.tensor_mul(
                            zh[j][:, ts(bc, BC)], z_t[j][:, ts(bc, BC)],
                            U[:, 4 + j, ts(bc, BC)],
                        )
                        ps_h = mm_group(8 + j, bc * BC, BC, 4, KT)
                        ps_i = mm_group(8 + j, bc * BC, BC, 0, 4)
                        nt = make_nt(j, bc * BC, BC, ps_i, ps_h, slice(0, BC))
                        combine(j, bc * BC, BC, nt, slice(0, BC))
                    else:
                        # final group: run the z gate LAST so the chain
                        # trailing the final matmul is the short
                        # sigmoid->zn->m->o path (tanh runs under z matmuls),
                        # split in halves so the chain pipelines
                        ps_h = mm_group(8 + j, bc * BC, BC, 4, KT)
                        ps_i = mm_group(8 + j, bc * BC, BC, 0, 4)
                        nt = make_nt(j, bc * BC, BC, ps_i, ps_h, slice(0, BC))
                        ps_z = mm_group(4 + j, bc * BC, BC, 0, KT)
                        HW_ = BC // 2
                        for s in range(2):
                            c0 = bc * BC + s * HW_
                            sl = slice(s * HW_, (s + 1) * HW_)
                            nc.scalar.activation(
                                z_t[j][:, c0 : c0 + HW_], ps_z[:, sl],
                                AFT.Sigmoid, bias=brz_sb[:, 4 + j : 5 + j],
                            )
                            nc.vector.tensor_mul(
                                zh[j][:, c0 : c0 + HW_], z_t[j][:, c0 : c0 + HW_],
                                U[:, 4 + j, c0 : c0 + HW_],
                            )
                            combine(j, c0, HW_, nt, sl)

    nc.compile()
    return nc


_SLOT_TO_G = [g for j in range(4) for g in (j, 4 + j, 8 + j)]


def _prep_core_inputs(x16, h16, W_ih16, W_hh16, b_ih, b_hh, n):
    bf16 = x16.dtype
    a_full = np.concatenate([W_ih16[n].T, W_hh16[n].T], axis=0)      # (1024, 1536)
    a_re = np.ascontiguousarray(
        a_full.reshape(KT, P, GT, P).transpose(2, 1, 0, 3)[_SLOT_TO_G]
        .transpose(1, 0, 2, 3)
        .reshape(P, GT * KT, P)
    )                                                                # (P, GT*KT, P)
    u = np.ascontiguousarray(
        np.concatenate(
            [x16[:, n * BS : (n + 1) * BS].T, h16[:, n * BS : (n + 1) * BS].T],
            axis=0,
        ).reshape(KT, P, B).transpose(1, 0, 2)
    )                                                                # (P, KT, B)
    brz8 = (b_ih[n, : 2 * BS] + b_hh[n, : 2 * BS]).reshape(8, P).T   # (P, 8)
    brz = np.ascontiguousarray(
        np.concatenate([brz8, -brz8[:, 4:8]], axis=1)
    )                                                                # (P, 12)
    bn = np.ascontiguousarray(
        np.concatenate(
            [b_ih[n, 2 * BS :].reshape(4, P).T, b_hh[n, 2 * BS :].reshape(4, P).T],
            axis=1,
        )
    )                                                                # (P, 8)
    return {"a": a_re, "u": u, "brz": brz, "bn": bn}


def kernel(x, h, W_ih, W_hh, b_ih, b_hh):
    global LAST_RESULTS
    import ml_dtypes

    bf16 = np.dtype(ml_dtypes.bfloat16)
    x16 = np.asarray(x, dtype=np.float32).astype(bf16)
    h16 = np.asarray(h, dtype=np.float32).astype(bf16)
    W_ih16 = np.asarray(W_ih, dtype=np.float32).astype(bf16)
    W_hh16 = np.asarray(W_hh, dtype=np.float32).astype(bf16)
    b_ih = np.asarray(b_ih, dtype=np.float32)
    b_hh = np.asarray(b_hh, dtype=np.float32)

    if "nc" not in _cache:
        _cache["nc"] = _build_nc()
    nc = _cache["nc"]

    in_maps = [
        _prep_core_inputs(x16, h16, W_ih16, W_hh16, b_ih, b_hh, n)
        for n in range(NB)
    ]
    trace = os.environ.get("BASS_KERNEL_TRACE") == "1"
    res = run_bass_kernel_spmd(nc, in_maps, list(range(NB)), trace=trace)
    LAST_RESULTS = res
    return np.concatenate(
        [res.results[n]["o"].astype(np.float32).T for n in range(NB)], axis=1
    )


# revision 7
# speedup vs baseline: 1.0778x; 1.0778x over previous
"""Block-diagonal GRU cell on 8 TRN2 NeuronCores — one block per core.

Math per block n (torch GRUCell):
  gi = x_n @ W_ih[n].T + b_ih[n]        (B, 3*BS)
  gh = h_n @ W_hh[n].T + b_hh[n]
  r = sigmoid(gi_r + gh_r); z = sigmoid(gi_z + gh_z)
  ng = tanh(gi_n + r * gh_n)
  h' = ng + z * (h_n - ng)

On-chip layout (per core): everything transposed on host so the
contraction (feature) dim is the SBUF partition dim and gates land on
PSUM partitions — biases then apply as per-partition ACT/DVE operands.
  A  = [W_ih[n].T ; W_hh[n].T]  -> (1024 feat, 1536 gates) in bf16,
       blocked per 128-gate column group, dram laid out partition-major
       so every slot-range load is one big contiguous-per-partition DMA.
  U  = [x_n.T ; h_n.T]          -> (1024 feat, 1024 batch) bf16
  out = h'.T                    -> (512, 1024) bf16, un-transposed and
       upcast on host.
All matmuls run in bf16 (full-rate PE, cheap LDWEIGHTS, half the HBM
traffic of fp32r). r/z gates accumulate x- and h-matmuls into one PSUM
bank (8 k-steps); the n gate keeps i_n / h_n in separate banks.
Combine avoids 1-z entirely: h' = nt - z*nt + z*h, with bf16
SBUF-resident DVE ops (2x dve mode) for everything not reading PSUM.
Loads are spread over three DGE queues (A on Sync, U on GpSimd SWDGE,
biases on Scalar) so trigger issue overhead doesn't serialize the fill;
short 128-wide PE warm-up matmuls bridge the fill for the p-state ramp.
"""

import os
import sys

import numpy as np

try:
    import concourse.bass as bass
except ImportError:  # fresh grading dir: fall back to the repo checkout
    sys.path.insert(0, "/opt/trn_rl_repo")
    import concourse.bass as bass

import concourse.mybir as mybir
import concourse.tile as tile
from concourse import bacc
from concourse.bass import ts
from concourse.bass_utils import run_bass_kernel_spmd

B = 1024            # batch
NB = 8              # blocks == cores
BS = 512            # hidden block size
G3 = 3 * BS         # gates per block (r, z, n)
KF = 1024           # contraction feats per core: 512 input + 512 hidden
P = 128
KT = KF // P        # 8 k-tiles
GT = G3 // P        # 12 gate column groups: 0-3 r, 4-7 z, 8-11 n
NBC = 2             # batch chunks
BC = B // NBC       # 512 (one PSUM bank of fp32)

F32 = mybir.dt.float32
BF16 = mybir.dt.bfloat16
AFT = mybir.ActivationFunctionType
ALU = mybir.AluOpType

_cache: dict = {}
LAST_RESULTS = None  # BassKernelResults of the most recent run (for test.py)


def _build_nc():
    nc = bacc.Bacc("TRN2", target_bir_lowering=False, debug=False, num_devices=NB)
    a_d = nc.dram_tensor("a", [P, GT * KT, P], BF16, kind="ExternalInput").ap()
    u_d = nc.dram_tensor("u", [P, KT, B], BF16, kind="ExternalInput").ap()
    brz_d = nc.dram_tensor("brz", [P, 12], F32, kind="ExternalInput").ap()
    bn_d = nc.dram_tensor("bn", [P, 8], F32, kind="ExternalInput").ap()
    o_d = nc.dram_tensor("o", [BS, B], BF16, kind="ExternalOutput").ap()

    with tile.TileContext(nc) as tc:
        with (
            tc.tile_pool(name="persist", bufs=1) as persist,
            tc.tile_pool(name="tmp", bufs=3) as tmp,
            tc.tile_pool(name="outp", bufs=4) as outp,
            tc.tile_pool(name="psum", bufs=8, space="PSUM") as psum,
        ):
            # tiny bias loads on the Scalar engine's queue, off the bulk paths
            brz_sb = persist.tile([P, 12], F32, name="brz_sb")
            nc.scalar.dma_start(brz_sb[:], brz_d[:])
            bn_sb = persist.tile([P, 8], F32, name="bn_sb")
            nc.scalar.dma_start(bn_sb[:], bn_d[:])

            U = persist.tile([P, KT, B], BF16, name="U")
            A = persist.tile([P, GT * KT, P], BF16, name="A")

            # PE warm-up scratch first on gpsimd so it's ready immediately
            wsb = persist.tile([P, P], BF16, name="wsb")
            nc.gpsimd.memset(wsb[:], 0.0)

            # Bulk loads, in consumption order. The Sync HWDGE queue starts
            # ~3us earlier than the GpSimd SWDGE queue, so everything the
            # first row-block group consumes rides Sync in exact order; the
            # late-needed bulk (U bc1, A slots for j>=1) rides GpSimd.
            def load_a(eng, s0, s1):
                eng.dma_start(
                    A[:, s0 * KT : s1 * KT, :], a_d[:, s0 * KT : s1 * KT, :]
                )

            def load_u(eng, k0, k1, bc):
                eng.dma_start(
                    U[:, k0:k1, ts(bc, BC)], u_d[:, k0:k1, ts(bc, BC)]
                )

            nc.sync.dma_start(A[:, 0:4, :], a_d[:, 0:4, :])      # r0 k0-3
            load_u(nc.sync, 0, 2, 0)
            nc.sync.dma_start(A[:, 4:KT, :], a_d[:, 4:KT, :])    # r0 k4-7
            load_u(nc.sync, 2, 4, 0)
            load_u(nc.sync, 4, 6, 0)
            load_a(nc.sync, 1, 2)       # z0
            load_u(nc.sync, 6, 8, 0)
            load_a(nc.sync, 2, 3)       # n0
            load_a(nc.sync, 3, 6)       # j=1
            load_a(nc.sync, 9, 12)      # j=3
            load_a(nc.gpsimd, 6, 9)     # j=2 (slow queue: needed ~20us in)
            load_u(nc.gpsimd, 0, 8, 1)  # U bc1 (needed ~32us in)

            # PE warm-up: short 128-wide matmuls bridge the DMA fill so the
            # HAM activity window is hot when real matmuls start
            wps = psum.tile([P, BC], F32, name="wps", tag="ps")
            for _ in range(10):
                nc.tensor.matmul(wps[:, :P], wsb[:], wsb[:], start=True, stop=True)

            # logical gate group -> A slot: slot 3j=r_j (g=j), 3j+1=z_j
            # (g=4+j), 3j+2=n_j (g=8+j)
            def slot_of(g):
                j, kind = g % 4, g // 4
                return 3 * j + kind

            def lhsT(g, k):
                return A[:, slot_of(g) * KT + k, :]

            # persistent per row-block j: r gate, z gate, zh = z*h (bf16)
            r_t = [persist.tile([P, B], BF16, name=f"r{j}") for j in range(4)]
            z_t = [persist.tile([P, B], BF16, name=f"z{j}") for j in range(4)]
            zh = [persist.tile([P, B], BF16, name=f"zh{j}") for j in range(4)]

            def mm_group(g, c0, w, k0, k1):
                ps = psum.tile([P, w], F32, name="ps", tag="ps")
                for k in range(k0, k1):
                    nc.tensor.matmul(
                        ps[:],
                        lhsT(g, k),
                        U[:, k, c0 : c0 + w],
                        start=(k == k0),
                        stop=(k == k1 - 1),
                    )
                return ps

            def make_nt(j, c0, w, ps_i, ps_h, sl):
                # ng = tanh(i_n + b_in + r*(h_n + b_hn))
                t = tmp.tile([P, w], F32, name="t", tag="t")
                nc.vector.scalar_tensor_tensor(
                    t[:], ps_h[:, sl], bn_sb[:, 4 + j : 5 + j],
                    r_t[j][:, c0 : c0 + w], ALU.add, ALU.mult,
                )
                t2 = tmp.tile([P, w], BF16, name="t2", tag="t2")
                nc.vector.tensor_add(t2[:], t[:], ps_i[:, sl])
                nt = tmp.tile([P, w], BF16, name="nt", tag="nt")
                nc.scalar.activation(nt[:], t2[:], AFT.Tanh, bias=bn_sb[:, j : j + 1])
                return nt

            def combine(j, c0, w, nt, ntsl):
                # h' = ng - z*ng + z*h
                zn = tmp.tile([P, w], BF16, name="zn", tag="zn")
                nc.vector.tensor_mul(zn[:], z_t[j][:, c0 : c0 + w], nt[:, ntsl])
                m = tmp.tile([P, w], BF16, name="m", tag="m")
                nc.vector.tensor_sub(m[:], nt[:, ntsl], zn[:])
                o_t = outp.tile([P, w], BF16, name="o_t", tag="o_t")
                nc.vector.tensor_add(o_t[:], m[:], zh[j][:, c0 : c0 + w])
                nc.sync.dma_start(o_d[ts(j, P), c0 : c0 + w], o_t[:])

            for bc in range(NBC):
                for j in range(4):
                    last = bc == NBC - 1 and j == 3
                    ps_r = mm_group(j, bc * BC, BC, 0, KT)
                    nc.scalar.activation(
                        r_t[j][:, ts(bc, BC)], ps_r[:], AFT.Sigmoid,
                        bias=brz_sb[:, j : j + 1],
                    )
                    if not last:
                        ps_z = mm_group(4 + j, bc * BC, BC, 0, KT)
                        nc.scalar.activation(
                            z_t[j][:, ts(bc, BC)], ps_z[:], AFT.Sigmoid,
                            bias=brz_sb[:, 4 + j : 5 + j],
                        )
                        nc.vector.tensor_mul(
                            zh[j][:, ts(bc, BC)], z_t[j][:, ts(bc, BC)],
                            U[:, 4 + j, ts(bc, BC)],
                        )
                        ps_h = mm_group(8 + j, bc * BC, BC, 4, KT)
                        ps_i = mm_group(8 + j, bc * BC, BC, 0, 4)
                        nt = make_nt(j, bc * BC, BC, ps_i, ps_h, slice(0, BC))
                        combine(j, bc * BC, BC, nt, slice(0, BC))
                    else:
                        # final group: run the z gate LAST so the chain
                        # trailing the final matmul is the short
                        # sigmoid->zn->m->o path (tanh runs under z matmuls),
                        # split in halves so the chain pipelines
                        ps_h = mm_group(8 + j, bc * BC, BC, 4, KT)
                        ps_i = mm_group(8 + j, bc * BC, BC, 0, 4)
                        nt = make_nt(j, bc * BC, BC, ps_i, ps_h, slice(0, BC))
                        HW_ = BC // 2
                        for s in range(2):
                            c0 = bc * BC + s * HW_
                            sl = slice(s * HW_, (s + 1) * HW_)
                            ps_z = mm_group(4 + j, c0, HW_, 0, KT)
                            nc.scalar.activation(
                                z_t[j][:, c0 : c0 + HW_], ps_z[:],
                                AFT.Sigmoid, bias=brz_sb[:, 4 + j : 5 + j],
                            )
                            nc.vector.tensor_mul(
                                zh[j][:, c0 : c0 + HW_], z_t[j][:, c0 : c0 + HW_],
                                U[:, 4 + j, c0 : c0 + HW_],
                            )
                            combine(j, c0, HW_, nt, sl)

    nc.compile()
    return nc


_SLOT_TO_G = [g for j in range(4) for g in (j, 4 + j, 8 + j)]


def _prep_core_inputs(x16, h16, W_ih16, W_hh16, b_ih, b_hh, n):
    bf16 = x16.dtype
    a_full = np.concatenate([W_ih16[n].T, W_hh16[n].T], axis=0)      # (1024, 1536)
    a_re = np.ascontiguousarray(
        a_full.reshape(KT, P, GT, P).transpose(2, 1, 0, 3)[_SLOT_TO_G]
        .transpose(1, 0, 2, 3)
        .reshape(P, GT * KT, P)
    )                                                                # (P, GT*KT, P)
    u = np.ascontiguousarray(
        np.concatenate(
            [x16[:, n * BS : (n + 1) * BS].T, h16[:, n * BS : (n + 1) * BS].T],
            axis=0,
        ).reshape(KT, P, B).transpose(1, 0, 2)
    )                                                                # (P, KT, B)
    brz8 = (b_ih[n, : 2 * BS] + b_hh[n, : 2 * BS]).reshape(8, P).T   # (P, 8)
    brz = np.ascontiguousarray(
        np.concatenate([brz8, -brz8[:, 4:8]], axis=1)
    )                                                                # (P, 12)
    bn = np.ascontiguousarray(
        np.concatenate(
            [b_ih[n, 2 * BS :].reshape(4, P).T, b_hh[n, 2 * BS :].reshape(4, P).T],
            axis=1,
        )
    )                                                                # (P, 8)
    return {"a": a_re, "u": u, "brz": brz, "bn": bn}


def kernel(x, h, W_ih, W_hh, b_ih, b_hh):
    global LAST_RESULTS
    import ml_dtypes

    bf16 = np.dtype(ml_dtypes.bfloat16)
    x16 = np.asarray(x, dtype=np.float32).astype(bf16)
    h16 = np.asarray(h, dtype=np.float32).astype(bf16)
    W_ih16 = np.asarray(W_ih, dtype=np.float32).astype(bf16)
    W_hh16 = np.asarray(W_hh, dtype=np.float32).astype(bf16)
    b_ih = np.asarray(b_ih, dtype=np.float32)
    b_hh = np.asarray(b_hh, dtype=np.float32)

    if "nc" not in _cache:
        _cache["nc"] = _build_nc()
    nc = _cache["nc"]

    in_maps = [
        _prep_core_inputs(x16, h16, W_ih16, W_hh16, b_ih, b_hh, n)
        for n in range(NB)
    ]
    trace = os.environ.get("BASS_KERNEL_TRACE") == "1"
    res = run_bass_kernel_spmd(nc, in_maps, list(range(NB)), trace=trace)
    LAST_RESULTS = res
    return np.concatenate(
        [res.results[n]["o"].astype(np.float32).T for n in range(NB)], axis=1
    )
